# revision 1
# baseline (speedup 1.0000x reference)
"""Trainium2 8-core Bass kernel: out = sigmoid(encoder_outputs @ hidden),
encoder_outputs [32768, 1024] f32, hidden [1024] f32 -> [1, 1, 32768] f32.

Sharding: encoder_outputs splits along seq_len into 8 slices of [4096, 1024]
(one per NeuronCore); hidden is replicated; each core produces its 4096
sigmoid scores and the host concatenates. No collectives needed.

Per-core kernel (raw bacc, hand-placed semaphores; no Tile machinery):
  - partition p owns rows [32p, 32p+32) of the slice, so scores map to the
    output vector with per-partition contiguous stores
  - only hidden preloads via HWDGE (broadcast to 128 partitions); ALL 32
    encoder rows stream as full-width (128-partition) SWDGE cast-DMAs
    (f32 DRAM -> bf16 SBUF) with a smoothed taper [8,6,5,4,3,2,1,1,1,1].
    f32 HWDGE head rows were measured to cost the SDMA engines ~1.6x a
    cast row, so streaming everything is net faster. Full-width ops
    distribute descriptors uniformly across all 16 SDMA engines
    (partition-subset ops get chunked onto arbitrary engines and cannot
    steer work away from a slow engine, measured); SDMA engine 15 runs
    ~20% slower in most runs (probabilistic device mode), so the taper
    keeps late loads small to shorten the post-stream chain at its pace
  - per main load, one bf16 VectorEngine tensor_tensor (2x packed)
    multiplies its rows against hidden; the leading DVE_ROWS reduce on
    DVE (batched tensor_reduce), the rest accumulate on ScalarE
    (activation Copy + accum, ~1.4us/row serial), all into f32 scores --
    the split keeps ACT's backlog off the tail
  - rows 28-31 are single-row loads consumed by fused
    scalar_tensor_tensor ops (out = row * h, accum_out = row-sum in f32)
    pipelined with their arrivals, so the post-last-byte chain is just
    sem-receipt -> STT -> sigmoid -> store; the final store issues from
    ScalarE's HWDGE ring to skip a sync-engine hop
  - stores are fire-and-forget (sem increments satisfy walrus codegen but
    nothing waits them): the ~8us NRT epilogue provides the flush window,
    so the end barrier is not gated on the last HBM write receipt
  - a warm Sigmoid on the const-zero AP pulls the single ACT funcset load
    off the tail; branch hints arm each engine's end-block branch
  - the Bass.__init__ post-memset all-engine barrier is deleted from the
    preamble IR (the NEFF-level preamble barrier already synchronizes
    engines; the only pre-stream const consumer is the warm sigmoid,
    whose output is discarded) and the block-exit sem barrier is dropped
    (the NRT epilogue ring synchronizes before the semaphore clears):
    together the stream's first byte lands ~1.9us earlier (8.5us vs
    10.4us, measured)
Known fixed costs inside the measured window: ~2.4us ramp (first_useful
anchors on the const-pool memsets, which sigmoid bias_ptr requires;
barrier deletion brought first byte to ~8.4us) and a ~7us NRT-injected
epilogue (253-semaphore clear + barriers). CAUTION: deleting further
preamble instructions (e.g. the 3 unused const memsets) shifted IRAM
code layout and stalled the first descriptor-gen by ~2.3us, a net
regression -- do not slim the preamble further without re-measuring.
The ~2.3us SWDGE doorbell-to-first-byte gap is per-transfer HBM latency,
not one-time ring init (a primer op does not shrink it, measured). Per-SDMA-engine read rate is ~25.7 GB/s for both
cast and plain DMAs -- the stream is at that floor. tensor_tensor_reduce
faults on this silicon; use scalar_tensor_tensor for fused
multiply+reduce. bf16 keeps rel err ~5.4e-3 (gate 2e-2). Measured
55.7/55.6us (uniform mode), ~62.7-65us (slow mode) vs 73.4us baseline.
"""
import numpy as np
from concourse.bass_utils import run_bass_kernel_spmd


import concourse.bass as bass
from concourse import bacc, mybir


class _HintedBlock(bass.BassBlock):
    """no_gpsimd_drain block whose end-bb branches carry prefetch hints."""

    def __init__(self, bass_, name):
        super().__init__(bass_, name, no_gpsimd_drain=True)
        self.hint_locs = {}

    def __exit__(self, exc_type, exc_val, exc_tb):
        if exc_type is not None:
            return
        for engine, last_body in self.last_body.items():
            with self.bass.body(last_body, parent=self.bass.cur_bb,
                                allow_existing_parent=True):
                br = engine.br(self.end_bb)
                loc = self.hint_locs.get(engine)
                if loc is not None:
                    br.branch_hint(loc)
        self.bass.switch_bb(self.end_bb)
        gpsimd_type = self.bass.gpsimd.engine
        for eng_type, eng in self.bass.engines.items():
            if eng_type == gpsimd_type:
                continue
            d = mybir.InstDrain(
                name=self.bass.get_next_instruction_name(),
                ins=[], outs=[], bass_is_fusable=False)
            d.engine = eng_type
            eng.add_instruction(d)

N_CORES = 8
SEQ = 32768
D = 1024
ROWS = SEQ // N_CORES          # 4096
RPP = ROWS // 128              # 32
F32 = mybir.dt.float32
BF16 = mybir.dt.bfloat16

HEAD_ROWS = 0                    # all rows stream (f32 head rows cost the
# slow SDMA engine ~2.4us/row vs ~1.5us/row cast; only hidden preloads)
MAIN_SIZES = [8, 6, 5, 4, 3, 2]  # rows 0..27: TT + DVE/ACT reduce split
DVE_ROWS = [3, 2, 2, 2, 1, 0]    # leading rows per load reduced on DVE
N_TAIL = 4                       # rows 28-31: single-row loads, each a
# fused scalar_tensor_tensor straight into scores as it arrives
SIG1 = HEAD_ROWS + sum(MAIN_SIZES)   # 28: first sigmoid covers cols < 28
assert SIG1 + N_TAIL == RPP


def build():
    nc = bacc.Bacc("TRN2", target_bir_lowering=False, debug=False,
                   num_devices=N_CORES)
    _entry = nc.m.functions[0].blocks[0].instructions
    _last_ms = max(i for i, x in enumerate(_entry)
                   if isinstance(x, mybir.InstMemset))
    del _entry[_last_ms + 1:]
    h_dram = nc.dram_tensor("hidden", [D], F32, kind="ExternalInput")
    e_dram = nc.dram_tensor("encoder_outputs", [ROWS, D], F32,
                            kind="ExternalInput")
    o_dram = nc.dram_tensor("out", [ROWS], F32, kind="ExternalOutput")
    ev3 = e_dram.ap().rearrange("(p r) d -> p r d", p=128)   # [128, 32, D]
    o_rear = o_dram.ap().rearrange("(p r) -> p r", p=128)    # [128, 32]

    eall = nc.alloc_sbuf_tensor("eall", [128, RPP * D], BF16)
    htf = nc.alloc_sbuf_tensor("htf", [128, D], F32)
    ht = nc.alloc_sbuf_tensor("ht", [128, D], BF16)
    prods = [nc.alloc_sbuf_tensor(f"prod{i}", [128, sz * D], BF16)
             for i, sz in enumerate(MAIN_SIZES)]
    scores = nc.alloc_sbuf_tensor("scores", [128, RPP], F32)
    sig = nc.alloc_sbuf_tensor("sigout", [128, RPP], F32)

    h_sem = nc.alloc_semaphore("hld")
    n_loads = len(MAIN_SIZES) + N_TAIL     # main + three 1-row tail loads
    lsems = [nc.alloc_semaphore(f"l{i}") for i in range(n_loads)]
    tt_sem = nc.alloc_semaphore("tt")      # DVE tensor_tensor completions
    row_sem = nc.alloc_semaphore("row")    # rows 0..SIG1-1 completions
    trow_sem = nc.alloc_semaphore("trow")  # rows SIG1..31 completions
    sig_sem = nc.alloc_semaphore("sg")
    outd_sem = nc.alloc_semaphore("outd")

    main_r0 = np.cumsum([HEAD_ROWS] + MAIN_SIZES)  # first row of each load

    def eslot(r0, r1):
        return eall.ap()[:, (r0 - HEAD_ROWS) * D:(r1 - HEAD_ROWS) * D]

    with _HintedBlock(nc, f"blk{nc.next_id()}") as block:

        @block.gpsimd
        def _(g: bass.BassEngine):
            block.hint_locs[g] = g.mark_branch_hint_location()
            spans = [(int(main_r0[i]), int(main_r0[i]) + sz)
                     for i, sz in enumerate(MAIN_SIZES)]
            spans += [(r, r + 1) for r in range(SIG1, RPP)]
            for i, (r0, r1) in enumerate(spans):
                g.dma_start(
                    out=eslot(r0, r1),
                    in_=ev3[:, r0:r1, :].rearrange("p r d -> p (r d)"),
                ).then_inc(lsems[i], 16)

        @block.vector
        def _(v: bass.BassEngine):
            block.hint_locs[v] = v.mark_branch_hint_location()
            v.wait_ge(h_sem, 16)
            v.tensor_copy(out=ht.ap(), in_=htf.ap())

            def tt_batch(dst, src, sz, hvec):
                return v.tensor_tensor(
                    out=dst.rearrange("p (r d) -> p r d", r=sz),
                    in0=src.rearrange("p (r d) -> p r d", r=sz),
                    in1=hvec.unsqueeze(1).broadcast_to((128, sz, D)),
                    op=mybir.AluOpType.mult,
                )

            # main loads: batched TT; DVE reduces the leading DVE_ROWS[i]
            # rows (batched), ACT accumulates the rest
            for i, sz in enumerate(MAIN_SIZES):
                r0 = int(main_r0[i])
                dr = DVE_ROWS[i]
                v.wait_ge(lsems[i], 16)
                tt_batch(prods[i].ap(), eslot(r0, r0 + sz), sz,
                         ht.ap()).then_inc(tt_sem, 1)
                if dr:
                    v.tensor_reduce(
                        out=scores.ap()[:, r0:r0 + dr],
                        in_=prods[i].ap()[:, 0:dr * D].rearrange(
                            "p (r d) -> p r d", r=dr),
                        axis=mybir.AxisListType.X, op=mybir.AluOpType.add,
                    ).then_inc(row_sem, dr)
            # rows 29-31: fused multiply+reduce pipelined with arrivals
            for k, r in enumerate(range(SIG1, RPP)):
                v.wait_ge(lsems[len(MAIN_SIZES) + k], 16)
                v.scalar_tensor_tensor(
                    out=eslot(r, r + 1), in0=eslot(r, r + 1),
                    scalar=1.0, in1=ht.ap(),
                    op0=mybir.AluOpType.mult, op1=mybir.AluOpType.mult,
                    accum_out=scores.ap()[:, r:r + 1],
                ).then_inc(trow_sem, 1)

        @block.scalar
        def _(s: bass.BassEngine):
            block.hint_locs[s] = s.mark_branch_hint_location()
            # warm the sigmoid funcset off the critical tail
            cz = nc.const_aps.scalar_like(0.0, sig.ap()[:, 0:1])
            s.activation(out=sig.ap()[:, 0:1], in_=cz,
                         func=mybir.ActivationFunctionType.Sigmoid)

            def accum(src, col, sem):
                return s.activation(
                    out=src, in_=src,
                    func=mybir.ActivationFunctionType.Copy,
                    accum_out=scores.ap()[:, col:col + 1],
                ).then_inc(sem, 1)

            for i, sz in enumerate(MAIN_SIZES):
                r0 = int(main_r0[i])
                s.wait_ge(tt_sem, 1 + i)
                for j in range(DVE_ROWS[i], sz):
                    accum(prods[i].ap()[:, j * D:(j + 1) * D], r0 + j,
                          row_sem)
            s.wait_ge(row_sem, SIG1)
            s.activation(
                out=sig.ap()[:, :SIG1], in_=scores.ap()[:, :SIG1],
                func=mybir.ActivationFunctionType.Sigmoid,
            ).then_inc(sig_sem, 1)
            s.wait_ge(trow_sem, RPP - SIG1)
            s.activation(
                out=sig.ap()[:, SIG1:], in_=scores.ap()[:, SIG1:],
                func=mybir.ActivationFunctionType.Sigmoid,
            )
            s.dma_start(out=o_rear[:, SIG1:],
                        in_=sig.ap()[:, SIG1:]).then_inc(outd_sem, 16)

        @block.sync
        def _(sy: bass.BassEngine):
            block.hint_locs[sy] = sy.mark_branch_hint_location()
            sy.dma_start(
                out=htf.ap(),
                in_=h_dram.ap().unsqueeze(0).broadcast_to((128, D))
            ).then_inc(h_sem, 16)
            sy.wait_ge(sig_sem, 1)
            sy.dma_start(out=o_rear[:, :SIG1],
                         in_=sig.ap()[:, :SIG1]).then_inc(outd_sem, 16)

    nc.compile()
    return nc


def make_in_maps(hidden, encoder_outputs):
    hidden = np.ascontiguousarray(np.asarray(hidden, dtype=np.float32))
    encoder_outputs = np.asarray(encoder_outputs, dtype=np.float32)
    return [
        {"hidden": hidden,
         "encoder_outputs": np.ascontiguousarray(
             encoder_outputs[i * ROWS:(i + 1) * ROWS])}
        for i in range(N_CORES)
    ]


_NC_CACHE = None


def _get_nc():
    global _NC_CACHE
    if _NC_CACHE is None:
        _NC_CACHE = build()
    return _NC_CACHE


def _make_in_maps(hidden, encoder_outputs):
    return make_in_maps(hidden, encoder_outputs)


def kernel(hidden, encoder_outputs):
    nc = _get_nc()
    in_maps = make_in_maps(hidden, encoder_outputs)
    res = run_bass_kernel_spmd(nc, in_maps, core_ids=list(range(N_CORES)))
    out = np.concatenate(
        [np.asarray(res.results[i]["out"]).reshape(-1) for i in range(N_CORES)])
    return out[None, None, :].astype(np.float32)

